# revision 15
# baseline (speedup 1.0000x reference)
"""AgentAttention Trainium2 kernel (v2, bf16).

Data-parallel over batch: 32 samples -> 8 cores x 4 samples.
Device layout is channels-major ("transposed"): activations live as (c, t).

Per-sample pipeline (all on device, bf16 matmul operands, fp32 PSUM):
  qk^T  = Wqk^T.T @ xs^T                  (bf16, FWL stationaries)
  v_t   = xs^T.T @ Wv^T   (tokens-major, for A1V; ones col for denominators)
  v^T   = Wv^T.T @ xs^T   (channels-major, zero-padded 34x34 image for dwc)
  agents^T: two-stage strided-window sums of q^T on DVE (contig inner axis)
  S1^T[t,(h,a)] = k^T.T @ blockdiag(agents) -> exp on ACT -> *expB1 (DVE 2x)
  A1V: agent_v + denominators via ones column; normalize on ACT (per-part scale)
  S2[(h,a),t]  = blockdiag(agents).T @ q^T  -> exp on ACT -> *expB2 (DVE 2x)
  A2V: single matmul per pair; ones cols in stationary produce denominators
  normalize: 1 reciprocal per pair + gpsimd partition_broadcast (SBUF only)
  dwc: 9 diagonal matmuls over shifted views of padded v^T
  proj: Wp^T.T @ pre_proj
Host adds proj/dwc biases and restores (b, n+1, c) order.
"""

import numpy as np
import ml_dtypes

REPEAT = 0  # >0: wrap sample loop in a hardware For_i for timing
UNROLL = 1  # copies of the body inside the For_i loop (timing diagnostics)
BCAST_MODE = "dma0"  # "dma0" = SBUF->SBUF DMA broadcast, "dram" = DRAM bounce

import concourse.bacc as bacc
import concourse.tile as tile
import concourse.mybir as mybir
from concourse import bass_utils

N_CORES = 8
B = 32
SPB = B // N_CORES  # samples per core
C = 256
NT = 1024  # spatial tokens
WIN = 32
HEADS = 8
HD = 32
AGENT = 49
POOL = 7
SCALE = HD ** -0.5

F32 = mybir.dt.float32
F32R = mybir.dt.float32r
BF16 = mybir.dt.bfloat16
AF = mybir.ActivationFunctionType
ALU = mybir.AluOpType
AX = mybir.AxisListType

BINS_START = [(i * WIN) // POOL for i in range(POOL)]
BINS_END = [-((-(i + 1) * WIN) // POOL) for i in range(POOL)]


# ----------------------------------------------------------------- host prep
def _resize_bilinear_7_to_32(b):
    """jax.image.resize 'bilinear' (half-pixel) for trailing (7,7)->(32,32)."""
    src, dst = 7, 32
    coords = (np.arange(dst) + 0.5) * (src / dst) - 0.5
    i0 = np.floor(coords).astype(np.int64)
    frac = coords - i0
    i0c = np.clip(i0, 0, src - 1)
    i1c = np.clip(i0 + 1, 0, src - 1)

    def along(x, axis):
        a0 = np.take(x, i0c, axis=axis)
        a1 = np.take(x, i1c, axis=axis)
        sh = [1] * x.ndim
        sh[axis] = dst
        f = frac.reshape(sh)
        return a0 * (1.0 - f) + a1 * f

    return along(along(b, -2), -1)


def _host_consts(qkv_w, proj_w, proj_b, dwc_w, dwc_b,
                 an_bias, ah_bias, aw_bias, na_bias, ha_bias, wa_bias):
    c = {}
    c["wqk"] = np.ascontiguousarray(
        qkv_w[:2 * C].T.reshape(2, 128, 2 * C)).astype(ml_dtypes.bfloat16)
    c["wv"] = np.ascontiguousarray(
        qkv_w[2 * C:].T.reshape(2, 128, C)).astype(ml_dtypes.bfloat16)
    c["wp"] = np.ascontiguousarray(
        proj_w.T.reshape(2, 128, C)).astype(ml_dtypes.bfloat16)

    # stage-1 bias, exp'ed, layout (t, 256*g + 64*h'' + a), pads -> exp(0)=1
    pb1 = _resize_bilinear_7_to_32(an_bias).reshape(HEADS, AGENT, NT)
    pb2 = (ah_bias + aw_bias).reshape(HEADS, AGENT, NT)
    b1 = pb1 + pb2  # (H, A, t)
    eb1 = np.zeros((NT, 512), np.float32)
    for g in range(2):
        for hh in range(4):
            eb1[:, 256 * g + 64 * hh:256 * g + 64 * hh + AGENT] = \
                b1[4 * g + hh].T
    c["expB1"] = np.exp(eb1).reshape(NT // 128, 128, 512).astype(ml_dtypes.bfloat16)

    # stage-2 bias, exp'ed, layout [pair][64*e + a, t]; pad rows ZERO so
    # expS2 pad rows come out exactly 0 after the multiply.
    ab1 = _resize_bilinear_7_to_32(na_bias).reshape(HEADS, AGENT, NT)  # [h,a,t]
    ha = ha_bias.reshape(HEADS, AGENT, WIN)      # [h, a, r]
    wa = wa_bias.reshape(HEADS, AGENT, WIN)      # [h, a, w]
    b2 = (ab1.reshape(HEADS, AGENT, WIN, WIN)
          + ha[:, :, :, None] + wa[:, :, None, :]).reshape(HEADS, AGENT, NT)
    eb2 = np.zeros((4, 128, NT), np.float32)
    for p in range(4):
        for e in range(2):
            eb2[p, 64 * e:64 * e + AGENT] = np.exp(b2[2 * p + e])
    c["expB2"] = eb2.astype(ml_dtypes.bfloat16)

    # pooled-agent scale (fold pool mean + attention scale), replicated rows
    sz = np.array([BINS_END[i] - BINS_START[i] for i in range(POOL)], np.float32)
    sa = SCALE / (sz[:, None] * sz[None, :])  # [i, j]
    c["sa"] = np.broadcast_to(sa.reshape(1, AGENT), (128, AGENT)).astype(np.float32).copy()

    # dwc diagonal blocks: slot tap*2+chunk, tap = 3*dr+dc
    w3 = dwc_w.reshape(C, 3, 3).astype(np.float32)
    w3d = np.zeros((18, 128, 128), np.float32)
    for tap in range(9):
        dr, dc = tap // 3, tap % 3
        for ci in range(2):
            np.fill_diagonal(w3d[tap * 2 + ci], w3[128 * ci:128 * ci + 128, dr, dc])
    c["w3d"] = w3d.astype(ml_dtypes.bfloat16)

    # ones pattern: col 0 all-ones bf16 (v_t ones column + BDagv ones cols)
    ob = np.ones((128, 1), np.float32)
    c["ones1"] = ob.astype(ml_dtypes.bfloat16)

    # host-side output biases
    c["bias_cls"] = proj_b.astype(np.float32)
    c["bias_sp"] = (proj_b + proj_w @ dwc_b).astype(np.float32)
    return c


def _mm512(nc, out, lhsT, rhs, start, stop, n):
    """matmul with the moving free dim split at 512 elements."""
    for n0 in range(0, n, 512):
        n1 = min(n0 + 512, n)
        nc.tensor.matmul(out[:, n0:n1], lhsT, rhs[:, n0:n1],
                         start=start, stop=stop)


# ------------------------------------------------------------- device build
def build_nc():
    nc = bacc.Bacc("TRN2", target_bir_lowering=False, debug=False,
                   num_devices=N_CORES)
    dr = {}
    dr["xT"] = nc.dram_tensor("xT", (SPB, 2, 128, NT + 1), BF16,
                              kind="ExternalInput").ap()
    dr["wqk"] = nc.dram_tensor("wqk", (2, 128, 512), BF16, kind="ExternalInput").ap()
    dr["wv"] = nc.dram_tensor("wv", (2, 128, 256), BF16, kind="ExternalInput").ap()
    dr["wp"] = nc.dram_tensor("wp", (2, 128, 256), BF16, kind="ExternalInput").ap()
    dr["expB1"] = nc.dram_tensor("expB1", (8, 128, 512), BF16, kind="ExternalInput").ap()
    dr["expB2"] = nc.dram_tensor("expB2", (4, 128, NT), BF16, kind="ExternalInput").ap()
    dr["sa"] = nc.dram_tensor("sa", (128, AGENT), F32, kind="ExternalInput").ap()
    dr["w3d"] = nc.dram_tensor("w3d", (18, 128, 128), BF16, kind="ExternalInput").ap()
    dr["ones1"] = nc.dram_tensor("ones1", (128, 1), BF16, kind="ExternalInput").ap()
    if BCAST_MODE == "dram":
        dr["scr"] = nc.dram_tensor("scr", (SPB, 4, 2, NT), F32, kind="Internal").ap()
    dr["y"] = nc.dram_tensor("y", (SPB, 2, 128, NT + 1), BF16,
                             kind="ExternalOutput").ap()

    with tile.TileContext(nc) as tc:
        _emit(tc, dr)
    nc.compile()
    return nc


def _emit(tc, dr):
    nc = tc.nc
    from contextlib import ExitStack
    with ExitStack() as ctx:
        cpool = ctx.enter_context(tc.tile_pool(name="consts", bufs=1))
        sp2 = ctx.enter_context(tc.tile_pool(name="sp2", bufs=2))
        sps = ctx.enter_context(tc.tile_pool(name="sps", bufs=2))
        spq = ctx.enter_context(tc.tile_pool(name="spq", bufs=3))
        ps_big = ctx.enter_context(tc.tile_pool(name="ps_big", bufs=2, space="PSUM"))
        ps_sm = ctx.enter_context(tc.tile_pool(name="ps_sm", bufs=2, space="PSUM"))
        ps_dwc = ctx.enter_context(tc.tile_pool(name="ps_dwc", bufs=2, space="PSUM"))

        # ---- constants to SBUF
        wqk = cpool.tile([128, 2, 512], BF16)
        wv = cpool.tile([128, 2, 256], BF16)
        wp = cpool.tile([128, 2, 256], BF16)
        eB1 = cpool.tile([128, 8, 512], BF16)
        eB2 = cpool.tile([128, 4, NT], BF16)
        sa = cpool.tile([128, AGENT], F32)
        w3d = cpool.tile([128, 18, 128], BF16)
        ones1 = cpool.tile([128, 1], BF16)
        for ki in range(2):
            nc.sync.dma_start(wqk[:, ki, :], dr["wqk"][ki])
            nc.sync.dma_start(wv[:, ki, :], dr["wv"][ki])
            nc.sync.dma_start(wp[:, ki, :], dr["wp"][ki])
        for ti in range(8):
            nc.sync.dma_start(eB1[:, ti, :], dr["expB1"][ti])
        for p in range(4):
            nc.sync.dma_start(eB2[:, p, :], dr["expB2"][p])
        nc.sync.dma_start(sa[:], dr["sa"][:])
        for s18 in range(18):
            nc.sync.dma_start(w3d[:, s18, :], dr["w3d"][s18])
        nc.sync.dma_start(ones1[:], dr["ones1"][:])

        # persistent tiles whose zero/ones regions are written exactly once.
        # Double-buffered (index s%2) so sample s+1's producers don't WAR-wait
        # on sample s's consumers.
        vTp = [cpool.tile([128, 2, 34, 34], BF16, name=f"vTp_{i}")
               for i in range(2)]
        BD1 = [cpool.tile([128, 2, 256], BF16, name=f"BD1_{i}")
               for i in range(2)]
        BD2 = [cpool.tile([128, 4, 128], BF16, name=f"BD2_{i}")
               for i in range(2)]
        BDagv = [cpool.tile([128, 4, 128], BF16, name=f"BDagv_{i}")
                 for i in range(2)]
        v_t = [cpool.tile([128, 8, 4, 65], BF16, name=f"v_t_{i}")
               for i in range(2)]
        eS1 = [cpool.tile([128, 8, 512], BF16, name=f"eS1_{i}")
               for i in range(2)]
        for b_ in range(2):
            # vTp: zero only the 1-pixel border ring (interior rewritten per sample)
            nc.gpsimd.memset(vTp[b_][:, :, 0, :], 0.0)
            nc.gpsimd.memset(vTp[b_][:, :, 33, :], 0.0)
            nc.gpsimd.memset(vTp[b_][:, :, 1:33, 0], 0.0)
            nc.gpsimd.memset(vTp[b_][:, :, 1:33, 33], 0.0)
            nc.gpsimd.memset(BD1[b_][:].rearrange("p a b -> p (a b)"), 0.0)
            nc.gpsimd.memset(BD2[b_][:].rearrange("p a b -> p (a b)"), 0.0)
            nc.gpsimd.memset(BDagv[b_][:].rearrange("p a b -> p (a b)"), 0.0)
            # eS1: zero only pad cols 49:64 of each 64-block (data rewritten per sample)
            nc.gpsimd.memset(
                eS1[b_][:].rearrange("p a (h c) -> p (a h) c", c=64)[:, :, 49:64], 0.0)
            # v_t ones column (col 64 of every (ti, p) block)
            for ti in range(8):
                for p4 in range(4):
                    nc.gpsimd.tensor_copy(v_t[b_][:, ti, p4, 64:65], ones1[:, 0:1])
            # BDagv denominator ones columns (constant across samples):
            #  even pairs: data cols 0:64;  ones col 64 (rows 0:49), col 96 (rows 64:113)
            #  odd  pairs: data cols 64:128; ones col 0 (rows 0:49), col 32 (rows 64:113)
            for p in range(0, 4, 2):
                nc.gpsimd.tensor_copy(BDagv[b_][0:49, p, 64:65], ones1[0:49, 0:1])
                nc.gpsimd.tensor_copy(BDagv[b_][64:113, p, 96:97], ones1[64:113, 0:1])
            for p in range(1, 4, 2):
                nc.gpsimd.tensor_copy(BDagv[b_][0:49, p, 0:1], ones1[0:49, 0:1])
                nc.gpsimd.tensor_copy(BDagv[b_][64:113, p, 32:33], ones1[64:113, 0:1])

        def body():
            pres = [None] * SPB
            for s in range(SPB):
                i = s % 2
                pres[s] = _sample(tc, dr, s, wqk, wv, wp, eB1, eB2, sa, w3d,
                                  vTp[i], BD1[i], BD2[i], BDagv[i], v_t[i], eS1[i],
                                  sp2, sps, spq, ps_big, ps_sm, ps_dwc)
                if s > 0:
                    _proj(tc, dr, s - 1, wp, pres[s - 1], sp2, ps_sm)
            _proj(tc, dr, SPB - 1, wp, pres[SPB - 1], sp2, ps_sm)

        if REPEAT > 0:
            with tc.For_i(0, REPEAT, 1):
                for _ in range(UNROLL):
                    body()
        else:
            body()


def _sample(tc, dr, s, wqk, wv, wp, eB1, eB2, sa, w3d,
            vTp, BD1, BD2, BDagv, v_t, expS1,
            sp2, sps, spq, ps_big, ps_sm, ps_dwc):
    nc = tc.nc

    # ---- load x^T (2 chunks of (128, 1025)); col 0 = cls token
    xT = sp2.tile([128, 2, NT + 1], BF16, tag="xT")
    for ci in range(2):
        nc.sync.dma_start(xT[:, ci, :], dr["xT"][s, ci])

    # ---- qk^T: 4 m-chunks (q: 0,1 / k: 2,3), accumulate over 2 k-chunks
    qkT = spq.tile([128, 4, NT], BF16, tag="qkT")
    for mi in range(4):
        acc = ps_big.tile([128, NT], F32, tag="big")
        for ki in range(2):
            _mm512(nc, acc, wqk[:, ki, 128 * mi:128 * mi + 128],
                   xT[:, ki, 1:NT + 1], ki == 0, ki == 1, NT)
        nc.scalar.activation(qkT[:, mi, :], acc[:], AF.Copy)

    # ---- v tokens-major (128t x 256c per chunk) -> bf16 (..., 4, 65), ones col
    for ti in range(8):
        acc = ps_sm.tile([128, 256], F32, tag="sm")
        for ki in range(2):
            nc.tensor.matmul(acc[:], xT[:, ki, 1 + 128 * ti:1 + 128 * ti + 128],
                             wv[:, ki, :], start=(ki == 0), stop=(ki == 1))
        nc.vector.tensor_copy(
            v_t[:, ti, :, 0:64], acc[:].rearrange("p (a b) -> p a b", a=4))

    # ---- v^T into zero-padded (34,34) image per chunk (copy on ACT)
    for ci in range(2):
        acc = ps_big.tile([128, NT], F32, tag="big")
        for ki in range(2):
            _mm512(nc, acc, wv[:, ki, 128 * ci:128 * ci + 128],
                   xT[:, ki, 1:NT + 1], ki == 0, ki == 1, NT)
        nc.scalar.activation(
            vTp[:, ci, 1:33, 1:33], acc[:].rearrange("p (h w) -> p h w", h=32),
            AF.Copy)

    # ---- adaptive pool of q^T -> agents^T (AG), scaled; contig inner reduces
    RP = sps.tile([128, 2, POOL, WIN], F32, tag="RP")   # [p, ci, j, h]
    AGf = sps.tile([128, 2, AGENT], F32, tag="AGf")
    AG = sps.tile([128, 2, AGENT], BF16, tag="AG")
    qv = qkT[:, 0:2, :].rearrange("p c (h w) -> p c h w", h=WIN)  # (128,2,h,w)
    # stage 1: reduce over w-bins (contiguous inner) -> RP[p, ci, j, h]
    for j in range(POOL):
        nc.vector.reduce_sum(RP[:, :, j, :],
                             qv[:, :, :, BINS_START[j]:BINS_END[j]], axis=AX.X)
    # stage 2: reduce over h-bins (contiguous inner) -> AGf[p, ci, 7i:7i+7]
    for i in range(POOL):
        nc.vector.reduce_sum(AGf[:, :, 7 * i:7 * i + 7],
                             RP[:, :, :, BINS_START[i]:BINS_END[i]], axis=AX.X)
    for ci in range(2):
        nc.vector.tensor_tensor(AG[:, ci, :], AGf[:, ci, :], sa[:], op=ALU.mult)

    # ---- block-diagonal agent tiles
    for g in range(2):
        for hh in range(4):
            nc.gpsimd.tensor_copy(
                BD1[32 * hh:32 * hh + 32, g, 64 * hh:64 * hh + AGENT],
                AG[32 * hh:32 * hh + 32, g, :])
    for p in range(4):
        b = 64 * (p % 2)  # partition base of this pair's q^T rows
        for e in range(2):
            nc.gpsimd.tensor_copy(
                BD2[b + 32 * e:b + 32 * e + 32, p, 64 * e:64 * e + AGENT],
                AG[b + 32 * e:b + 32 * e + 32, p // 2, :])

    # ---- stage 1 scores^T (t, (h,a)) + exp + bias factor (DVE bf16 2x)
    for ti in range(8):
        acc = ps_sm.tile([128, 512], F32, tag="sm")
        for g in range(2):
            nc.tensor.matmul(acc[:, 256 * g:256 * g + 256],
                             qkT[:, 2 + g, 128 * ti:128 * ti + 128],
                             BD1[:, g, :], start=True, stop=True)
        ev = expS1[:, ti, :].rearrange("p (h c) -> p h c", c=64)[:, :, 0:49]
        av = acc[:].rearrange("p (h c) -> p h c", c=64)[:, :, 0:49]
        nc.scalar.activation(ev, av, AF.Exp)
    evall = expS1[:].rearrange("p a (h c) -> p (a h) c", c=64)[:, :, 0:49]
    bvall = eB1[:].rearrange("p a (h c) -> p (a h) c", c=64)[:, :, 0:49]
    nc.vector.tensor_tensor(evall, evall, bvall, op=ALU.mult)

    # ---- stage 2 scores ((h,a), t) + exp + bias factor (DVE bf16 2x)
    expS2 = sp2.tile([128, 4, NT], BF16, tag="expS2")
    for p in range(4):
        b = 64 * (p % 2)
        acc = ps_big.tile([128, NT], F32, tag="big")
        _mm512(nc, acc, BD2[b:b + 64, p, :],
               qkT[b:b + 64, p // 2, :], True, True, NT)
        nc.scalar.activation(expS2[:, p, :], acc[:], AF.Exp)
    e2all = expS2[:].rearrange("p a b -> p (a b)")
    b2all = eB2[:].rearrange("p a b -> p (a b)")
    nc.vector.tensor_tensor(e2all, e2all, b2all, op=ALU.mult)

    # ---- A1V: agent_v (pair-local rows 64e+a) + denominators (ones col 64)
    rec = sps.tile([128, 4, 1], F32, tag="rec")
    for p in range(4):
        acc = ps_sm.tile([128, 65], F32, tag="sm")
        c0 = 256 * (p // 2) + 128 * (p % 2)
        for ti in range(8):
            nc.tensor.matmul(acc[:], expS1[:, ti, c0:c0 + 128],
                             v_t[:, ti, p, :], start=(ti == 0), stop=(ti == 7))
        nc.vector.reciprocal(rec[0:113, p, :], acc[:113, 64:65])
        for e in range(2):
            dcol = 64 * (p % 2) + 32 * e  # data col base in BDagv
            nc.scalar.mul(BDagv[64 * e:64 * e + 49, p, dcol:dcol + 32],
                          acc[64 * e:64 * e + 49, 32 * e:32 * e + 32],
                          rec[64 * e:64 * e + 49, p, :])

    # ---- dwc matmuls early: they only need vTp, and fill PE idle time
    # while pooling/BD/exp chains run on DVE/ACT/gpsimd.
    dwc_accs = []
    for ci in range(2):
        for hf in range(2):
            acc = ps_dwc.tile([128, 512], F32, tag="dwc")
            accv = acc[:].rearrange("p (h w) -> p h w", h=16)
            for tap in range(9):
                dr_, dc_ = tap // 3, tap % 3
                nc.tensor.matmul(accv[:],
                                 w3d[:, tap * 2 + ci, :],
                                 vTp[:, ci, dr_ + 16 * hf:dr_ + 16 * hf + 16,
                                     dc_:dc_ + 32],
                                 start=(tap == 0), stop=(tap == 8))
            dwc_accs.append(acc)

    # ---- A2V + normalization -> pre_proj cols 0:1024 (spatial), col 1024=cls
    pre = sp2.tile([128, 2, NT + 1], BF16, tag="pre")
    for ci in range(2):
        nc.gpsimd.tensor_copy(pre[:, ci, NT:NT + 1], xT[:, ci, 0:1])
    for p in range(4):
        b = 64 * (p % 2)               # data row base == pre row base
        dn0 = 64 if p % 2 == 0 else 0  # denom rows at dn0 (e0) and dn0+32 (e1)
        acc = ps_big.tile([128, NT], F32, tag="big")
        _mm512(nc, acc, BDagv[0:113, p, :], expS2[0:113, p, :], True, True, NT)
        r2 = sps.tile([128, NT], F32, tag="r2")
        rb = sps.tile([128, NT], F32, tag="rb")
        nc.vector.reciprocal(r2[dn0:dn0 + 33, :], acc[dn0:dn0 + 33, :])
        nc.gpsimd.dma_start(
            rb[b:b + 64, :],
            r2[dn0:dn0 + 33:32, :].unsqueeze(1).broadcast_to((2, 32, NT)))
        nc.vector.tensor_tensor(
            pre[b:b + 64, p // 2, 0:NT],
            acc[b:b + 64, :], rb[b:b + 64, :], op=ALU.mult)

    # ---- dwc accumulated on top of pre_proj spatial cols
    for ci in range(2):
        for hf in range(2):
            acc = dwc_accs[2 * ci + hf]
            nc.vector.tensor_tensor(pre[:, ci, 512 * hf:512 * hf + 512],
                                    acc[:], pre[:, ci, 512 * hf:512 * hf + 512],
                                    op=ALU.add)

    return pre


def _proj(tc, dr, s, wp, pre, sp2, ps_sm):
    nc = tc.nc
    outT = sp2.tile([128, 2, NT + 1], BF16, tag="outT")
    for mi in range(2):
        for n0, n1 in ((0, 512), (512, 1024), (1024, 1025)):
            acc = ps_sm.tile([128, 512], F32, tag="sm")
            for ki in range(2):
                nc.tensor.matmul(acc[:, 0:n1 - n0],
                                 wp[:, ki, 128 * mi:128 * mi + 128],
                                 pre[:, ki, n0:n1], start=(ki == 0), stop=(ki == 1))
            nc.scalar.activation(outT[:, mi, n0:n1], acc[:, 0:n1 - n0], AF.Copy)
        nc.sync.dma_start(dr["y"][s, mi], outT[:, mi, :])


# ---------------------------------------------------------------- execution
_CACHE = {}


def _get_nc():
    if "nc" not in _CACHE:
        _CACHE["nc"] = build_nc()
    return _CACHE["nc"]


def make_in_maps(x, consts):
    in_maps = []
    for c in range(N_CORES):
        xs = x[SPB * c:SPB * (c + 1)]  # (4, 1025, 256)
        xT = np.ascontiguousarray(xs.transpose(0, 2, 1)).reshape(SPB, 2, 128, NT + 1)
        in_maps.append({
            "xT": xT.astype(ml_dtypes.bfloat16),
            "wqk": consts["wqk"], "wv": consts["wv"], "wp": consts["wp"],
            "expB1": np.ascontiguousarray(consts["expB1"]),
            "expB2": np.ascontiguousarray(consts["expB2"]),
            "sa": consts["sa"], "w3d": consts["w3d"],
            "ones1": consts["ones1"],
        })
    return in_maps


def assemble(results, consts):
    out = np.empty((B, NT + 1, C), np.float32)
    for c in range(N_CORES):
        y = results[c]["y"].astype(np.float32).reshape(SPB, 2, 128, NT + 1)
        yT = y.transpose(0, 3, 1, 2).reshape(SPB, NT + 1, C)  # (s, t, c)
        out[SPB * c:SPB * (c + 1), 0] = yT[:, NT] + consts["bias_cls"]
        out[SPB * c:SPB * (c + 1), 1:] = yT[:, :NT] + consts["bias_sp"]
    return out


def kernel(x, qkv_w, proj_w, proj_b, dwc_w, dwc_b,
           an_bias, ah_bias, aw_bias, na_bias, ha_bias, wa_bias):
    x = np.asarray(x, np.float32)
    consts = _host_consts(np.asarray(qkv_w, np.float32), np.asarray(proj_w, np.float32),
                          np.asarray(proj_b, np.float32), np.asarray(dwc_w, np.float32),
                          np.asarray(dwc_b, np.float32), np.asarray(an_bias, np.float32),
                          np.asarray(ah_bias, np.float32), np.asarray(aw_bias, np.float32),
                          np.asarray(na_bias, np.float32), np.asarray(ha_bias, np.float32),
                          np.asarray(wa_bias, np.float32))
    nc = _get_nc()
    in_maps = make_in_maps(x, consts)
    res = bass_utils.run_bass_kernel_spmd(nc, in_maps,
                                          core_ids=list(range(N_CORES)))
    return assemble(res.results, consts)
